# revision 22
# baseline (speedup 1.0000x reference)
"""ALIGNNConv edge-gate kernel — v10: paired row-side gathers, col-sorted
slot order, big-chunk streams, split HWDGE rings.

Gathers are ~85% descriptor-latency-bound (measured: 2x bytes per desc =
+13% time), so the row side pairs edges with consecutive rows (r, r+1)
into one 512B descriptor against an overlapping pair table
nfp[n] = concat(nf[n], nf[n+1]). Greedy adjacent matching pairs ~37% of
edges (per-core measured), cutting row descriptors to ~63%. Pair slots
yield two edge columns after two PE transposes (layer 0/1), so there is
no column inflation: sections are laid out [pair slots | singles].
Col side stays per-edge (random indices cannot pair), but slots within
each region are ordered by col value so the col gather (83K descs/core
vs the row side's 58K) walks HBM mostly ascending (row-buffer hits).

ef/out stream in 4096-column chunks (1MB DMAs, ~33% faster than 256KB);
ef loads drain the sync/SP HWDGE ring while out stores use the ACT ring.

Measured on HW (8 cores, per-iteration): ~418-423us vs 437-454us for the
v5 baseline; rel err 0.0030 (bf16 compute, f32 psum).
"""

import os as _os

import numpy as np
import ml_dtypes

BF16 = ml_dtypes.bfloat16

N_NODES = 50000
N_EDGES = 640000
D = 128
N_CORES = 8
NODES_PAD = 51200
H = NODES_PAD // 2
GROUP_MAX = int(_os.environ.get("V8_GROUP_MAX", "1024"))
SCRATCH = int(_os.environ.get("V8_SCRATCH", "65536"))
STREAM_CHUNK = int(_os.environ.get("V8_STREAM_CHUNK", "4096"))


class Cfg:
    """Per-section pair-slot and single-slot capacities.

    Section column layout: [2*pair_cap cols | single_cap cols].
    pair_cap % 256 == 0 (one 512-bank = 2 pair subtiles),
    single_cap % 512 == 0. Slot space (gather idx space) per section is
    pair_cap + single_cap.
    """

    def __init__(self, pair_cap, single_cap):
        assert len(pair_cap) == len(single_cap) == 4
        for p, s in zip(pair_cap, single_cap):
            assert p % 256 == 0 and s % 512 == 0
        self.pair_cap = tuple(int(p) for p in pair_cap)
        self.single_cap = tuple(int(s) for s in single_cap)
        self.sec_cols = tuple(
            2 * p + s for p, s in zip(self.pair_cap, self.single_cap)
        )
        self.sec_coloff = tuple(sum(self.sec_cols[:s]) for s in range(5))
        self.e_slots = self.sec_coloff[4]
        self.sec_slots = tuple(
            p + s for p, s in zip(self.pair_cap, self.single_cap)
        )
        self.sec_slotoff = tuple(sum(self.sec_slots[:s]) for s in range(5))
        self.n_idx_slots = self.sec_slotoff[4]
        # groups: (kind, sec, coloff, slotoff, nslot) with nslot <= 1024
        self.groups = []
        for s in range(4):
            coloff = self.sec_coloff[s]
            slotoff = self.sec_slotoff[s]
            rem = self.pair_cap[s]
            while rem > 0:
                n = min(GROUP_MAX, rem)
                self.groups.append(("p", s, coloff, slotoff, n))
                coloff += 2 * n
                slotoff += n
                rem -= n
            rem = self.single_cap[s]
            while rem > 0:
                n = min(GROUP_MAX, rem)
                self.groups.append(("s", s, coloff, slotoff, n))
                coloff += n
                slotoff += n
                rem -= n


E_CORE = N_EDGES // N_CORES


def build_nc(cfg: Cfg, repeat: int = 1, variant: str = "full"):
    assert variant == "full"
    import concourse.mybir as mybir
    from concourse import bacc
    from concourse import library_config
    from concourse.tile import TileContext
    from concourse.tile_rust import add_dep_helper

    f32 = mybir.dt.float32
    bf16 = mybir.dt.bfloat16
    i16 = mybir.dt.int16

    nc = bacc.Bacc(
        "TRN2",
        target_bir_lowering=False,
        debug=False,
        num_swdge_queues=4,
        dynamic_dma_scratch_size=SCRATCH,
    )

    nfn = nc.declare_dram_parameter("nfn", [NODES_PAD, D], bf16, isOutput=False)
    nfp = nc.declare_dram_parameter("nfp", [NODES_PAD, 2 * D], bf16, isOutput=False)
    w = nc.declare_dram_parameter("w", [3 * D, D], bf16, isOutput=False)
    bvec = nc.declare_dram_parameter("bvec", [D, 1], f32, isOutput=False)
    ident = nc.declare_dram_parameter("ident", [D, D], bf16, isOutput=False)
    n_iw = cfg.n_idx_slots // 16
    n_cw = cfg.e_slots // 16
    idxr = nc.declare_dram_parameter("idxr", [D, n_iw], i16, isOutput=False)
    idxc = nc.declare_dram_parameter("idxc", [D, n_cw], i16, isOutput=False)
    eft = nc.declare_dram_parameter("eft", [D, cfg.e_slots], bf16, isOutput=False)
    outp = nc.declare_dram_parameter("out", [D, cfg.e_slots], bf16, isOutput=True)

    with TileContext(nc) as tc:
        with (
            tc.tile_pool(name="const", bufs=1) as cpool,
            tc.tile_pool(name="pps", bufs=4, space="PSUM") as pps,
            tc.tile_pool(name="tps", bufs=2, space="PSUM") as tps,
            tc.tile_pool(name="gat", bufs=6 if GROUP_MAX > 1024 else 8) as gpool,
            tc.tile_pool(name="trs", bufs=4) as trpool,
            tc.tile_pool(name="edg", bufs=4) as epool,
            tc.tile_pool(name="gsb", bufs=3) as gspool,
        ):
            nc.gpsimd.load_library(library_config.mlp)

            w1 = cpool.tile([D, D], bf16, name="w1")
            w2 = cpool.tile([D, D], bf16, name="w2")
            w3 = cpool.tile([D, D], bf16, name="w3")
            bia = cpool.tile([D, 1], f32, name="bia")
            idt = cpool.tile([D, D], bf16, name="idt")
            ira = cpool.tile([D, n_iw], i16, name="ira")
            ica = cpool.tile([D, n_cw], i16, name="ica")

            def emit_consts():
                nc.sync.dma_start(out=w1[:], in_=w[0:D, :])
                nc.sync.dma_start(out=w2[:], in_=w[D : 2 * D, :])
                nc.sync.dma_start(out=w3[:], in_=w[2 * D : 3 * D, :])
                nc.sync.dma_start(out=bia[:], in_=bvec[:, :])
                nc.sync.dma_start(out=idt[:], in_=ident[:, :])
                nc.sync.dma_start(out=ira[:], in_=idxr[:, :])
                nc.sync.dma_start(out=ica[:], in_=idxc[:, :])

            def emit_edges():
                CH = STREAM_CHUNK
                echunks = {}
                ochunks = {}

                def chunk_of(go):
                    c = go // CH
                    if c not in echunks:
                        coff = c * CH
                        n = min(CH, cfg.e_slots - coff)
                        et = epool.tile([D, n], bf16, name="etc")
                        nc.sync.dma_start(out=et[:], in_=eft[:, coff : coff + n])
                        echunks[c] = (et, coff, n)
                        ochunks[c] = epool.tile([D, n], bf16, name="otc")
                    et, coff, n = echunks[c]
                    return et, ochunks[c], go - coff

                def flush_chunks(upto):
                    done = [
                        c for c, (et, coff, n) in echunks.items()
                        if coff + n <= upto
                    ]
                    for c in sorted(done):
                        et, coff, n = echunks[c]
                        # out via the ACT-engine HWDGE ring: ef loads (sync/SP
                        # ring) and out stores drain through separate FIFOs
                        nc.scalar.dma_start(
                            out=outp[:, coff : coff + n], in_=ochunks[c]
                        )
                        del echunks[c], ochunks[c]

                state = {"prev": None, "qn": 0}

                def gather(out_t, tab, idx_sl, nslot, elem):
                    g = nc.gpsimd.dma_gather(
                        out_t, tab, idx_sl, nslot, nslot, elem,
                        queue_num=state["qn"] % 4,
                        single_packet=True,
                    )
                    if state["prev"] is not None:
                        add_dep_helper(
                            g.ins, state["prev"].ins, sync=False,
                            reason="swdge lane/queue alignment",
                        )
                    state["prev"] = g
                    state["qn"] += 1
                    return g

                for kind, sec, coloff, slotoff, nslot in cfg.groups:
                    rh, ch = sec >> 1, sec & 1
                    ncols = 2 * nslot if kind == "p" else nslot
                    kmax = nslot // D
                    # row-side gather (paired: 512B/desc; single: 256B/desc)
                    if kind == "p":
                        gr = gpool.tile([D, kmax, 2 * D], bf16, name="grp")
                        gather(
                            gr[:], nfp[rh * H : rh * H + H, :],
                            ira[:, slotoff // 16 : (slotoff + nslot) // 16],
                            nslot, 2 * D,
                        )
                    else:
                        gr = gpool.tile([D, kmax, D], bf16, name="grs1")
                        gather(
                            gr[:], nfn[rh * H : rh * H + H, :],
                            ira[:, slotoff // 16 : (slotoff + nslot) // 16],
                            nslot, D,
                        )
                    # col-side gathers: per 1024 edge columns
                    gcs_list = []
                    for h in range(0, ncols, GROUP_MAX):
                        nc_cols = min(GROUP_MAX, ncols - h)
                        gc = gpool.tile([D, nc_cols // D, D], bf16, name="gcol")
                        gather(
                            gc[:], nfn[ch * H : ch * H + H, :],
                            ica[:, (coloff + h) // 16 : (coloff + h + nc_cols) // 16],
                            nc_cols, D,
                        )
                        gcs_list.append((h, nc_cols, gc))

                    # compute per 512-col bank
                    EB = 4
                    for kb in range(ncols // (EB * D)):
                        cb = kb * EB * D  # col offset within group
                        go = coloff + cb
                        et, o_t, rel = chunk_of(go)
                        esl = slice(rel, rel + EB * D)
                        ps = pps.tile([D, EB * D], f32, name="ps")
                        grt = tps.tile([D, EB * D], bf16, name="grt")
                        gct = tps.tile([D, EB * D], bf16, name="gct")
                        # row-side transposes into grt
                        if kind == "p":
                            # bank covers 2 pair subtiles (k, k+1): each gives
                            # T0|T1 (256 cols)
                            for j in range(2):
                                k = kb * 2 + j
                                for lay in range(2):
                                    psl = slice(
                                        (2 * j + lay) * D, (2 * j + lay + 1) * D
                                    )
                                    nc.tensor.transpose(
                                        grt[:, psl],
                                        gr[:, k, lay * D : (lay + 1) * D],
                                        idt[:],
                                    )
                        else:
                            for k4 in range(EB):
                                k = kb * EB + k4
                                psl = slice(k4 * D, (k4 + 1) * D)
                                nc.tensor.transpose(
                                    grt[:, psl], gr[:, k, :], idt[:]
                                )
                        # col-side transposes
                        gh, gn, gc = next(
                            (h, n2, g) for h, n2, g in gcs_list
                            if h <= cb < h + n2
                        )
                        for k4 in range(EB):
                            kc = (cb - gh) // D + k4
                            psl = slice(k4 * D, (k4 + 1) * D)
                            nc.tensor.transpose(
                                gct[:, psl], gc[:, kc, :], idt[:]
                            )
                        grs = trpool.tile([D, EB * D], bf16, name="grs")
                        gcs = trpool.tile([D, EB * D], bf16, name="gcs")
                        nc.scalar.copy(grs[:], grt[:])
                        nc.vector.tensor_copy(gcs[:], gct[:])
                        nc.tensor.matmul(ps[:], w1[:], grs[:], start=True, stop=False)
                        nc.tensor.matmul(ps[:], w2[:], gcs[:], start=False, stop=False)
                        nc.tensor.matmul(ps[:], w3[:], et[:, esl], start=False, stop=True)
                        gate = gspool.tile([D, EB * D], bf16, name="gate")
                        nc.scalar.activation(
                            gate[:], ps[:],
                            mybir.ActivationFunctionType.Sigmoid,
                            bias=bia[:, 0:1],
                        )
                        nc.vector.tensor_mul(o_t[:, esl], et[:, esl], gate[:])
                    flush_chunks(coloff + ncols)
                flush_chunks(cfg.e_slots + 1)

            def emit_body():
                emit_consts()
                emit_edges()

            if repeat > 1:
                with tc.For_i(0, repeat, 1):
                    emit_body()
            else:
                emit_body()

    nc.finalize()
    return nc


def _match_section(r):
    """Greedy adjacent matching on relative rows r (values in [0, H)).

    Returns (pair_base_rows, pair_edge_a, pair_edge_b, single_edges):
    pair i joins edge_a[i] (row base) with edge_b[i] (row base+1);
    arrays are ordered by base row. single_edges ordered by row.
    """
    order = np.argsort(r, kind="stable")
    c = np.bincount(r, minlength=H)
    starts = np.zeros(H + 1, np.int64)
    starts[1:] = np.cumsum(c)
    k_prev = 0
    bases, pa, pb, singles = [], [], [], []
    for i in range(H):
        rem = c[i] - k_prev
        k = min(rem, c[i + 1]) if i + 1 < H else 0
        if k > 0:
            a = order[starts[i] + k_prev : starts[i] + k_prev + k]
            b = order[starts[i + 1] : starts[i + 1] + k]
            bases.append(np.full(k, i, np.int32))
            pa.append(a)
            pb.append(b)
        if rem - k > 0:
            singles.append(order[starts[i] + k_prev + k : starts[i + 1]])
        k_prev = k
    cat = lambda L, dt: (
        np.concatenate(L) if L else np.empty(0, dt)
    )
    return (
        cat(bases, np.int32),
        cat(pa, np.int64),
        cat(pb, np.int64),
        cat(singles, np.int64),
    )


def _wrap16(a):
    """[n] int16 -> [128, n//16] gather idx layout."""
    return np.ascontiguousarray(
        np.tile(a.reshape(-1, 16).T, (8, 1))
    )


def analyze(edge_index, n_cores):
    """Per-core per-section (pair, single) counts and the per-core layout."""
    ei = np.asarray(edge_index)
    e_core = ei.shape[1] // n_cores
    layouts = []
    for i in range(n_cores):
        sl = slice(i * e_core, (i + 1) * e_core)
        er = ei[0, sl].astype(np.int64)
        ec = ei[1, sl].astype(np.int64)
        sec = (er >= H).astype(np.int64) * 2 + (ec >= H)
        per_sec = []
        for s in range(4):
            idx = np.nonzero(sec == s)[0]
            r = (er[idx] - (s >> 1) * H).astype(np.int64)
            bases, pa, pb, singles = _match_section(r)
            per_sec.append((idx, r, bases, pa, pb, singles))
        layouts.append(per_sec)
    return layouts


def derive_cfg(edge_index, n_cores):
    layouts = analyze(edge_index, n_cores)
    pmax = np.zeros(4, np.int64)
    smax = np.zeros(4, np.int64)
    for per_sec in layouts:
        for s in range(4):
            _, _, bases, _, _, singles = per_sec[s]
            pmax[s] = max(pmax[s], len(bases))
            smax[s] = max(smax[s], len(singles))
    pair_cap = [max(256, -(-int(p) // 256) * 256) for p in pmax]
    single_cap = [max(512, -(-int(s) // 512) * 512) for s in smax]
    cfg = Cfg(pair_cap, single_cap)
    cfg.layouts = layouts
    return cfg


def make_in_maps(cfg: Cfg, node_features, edge_index, edge_features, W, b,
                 n_cores):
    layouts = cfg.layouts
    nf = np.asarray(node_features, dtype=np.float32)
    nf_pad = np.zeros((NODES_PAD, D), dtype=np.float32)
    nf_pad[: nf.shape[0]] = nf
    nfn = np.ascontiguousarray(nf_pad.astype(BF16))
    nfp = np.zeros((NODES_PAD, 2 * D), dtype=BF16)
    nfp[:, :D] = nfn
    nfp[:-1, D:] = nfn[1:]

    w_bf = np.ascontiguousarray(np.asarray(W, dtype=np.float32).astype(BF16))
    bv = np.asarray(b, dtype=np.float32).reshape(D, 1)
    ident = np.eye(D, dtype=BF16)

    ei = np.asarray(edge_index)
    ef = np.asarray(edge_features, dtype=np.float32)
    e_core = ei.shape[1] // n_cores

    in_maps, perms = [], []
    for i in range(n_cores):
        sl = slice(i * e_core, (i + 1) * e_core)
        ec = ei[1, sl].astype(np.int64)
        ef_bf = ef[sl].astype(BF16)
        per_sec = layouts[i]

        idx_slot = np.zeros(cfg.n_idx_slots, np.int16)  # row gather idxs
        colv = np.zeros(cfg.e_slots, np.int16)          # col gather idxs
        perm = np.full(cfg.e_slots, -1, np.int64)       # col -> edge id

        for s in range(4):
            idx, r, bases, pa, pb, singles = per_sec[s]
            ch = s & 1
            # order pair slots by layer-0 col and singles by col: the col
            # gather (83K descs/core, more than the row side's 58K) then
            # walks HBM mostly ascending (row-buffer hits); row-side
            # descriptors are latency-bound and less order-sensitive
            if len(bases):
                op = np.argsort(ec[idx[pa]], kind="stable")
                bases, pa, pb = bases[op], pa[op], pb[op]
            if len(singles):
                osg = np.argsort(ec[idx[singles]], kind="stable")
                singles = singles[osg]
            P, S = len(bases), len(singles)
            soff = cfg.sec_slotoff[s]
            coff = cfg.sec_coloff[s]
            # pair slots
            idx_slot[soff : soff + P] = bases.astype(np.int16)
            t = np.arange(P)
            col_a = coff + (t // 128) * 256 + (t % 128)
            col_b = col_a + 128
            ea = idx[pa]
            eb = idx[pb]
            perm[col_a] = ea
            perm[col_b] = eb
            colv[col_a] = (ec[ea] - ch * H).astype(np.int16)
            colv[col_b] = (ec[eb] - ch * H).astype(np.int16)
            # single slots
            s0 = soff + cfg.pair_cap[s]
            idx_slot[s0 : s0 + S] = r[singles].astype(np.int16)
            c0 = coff + 2 * cfg.pair_cap[s]
            es = idx[singles]
            perm[c0 : c0 + S] = es
            colv[c0 : c0 + S] = (ec[es] - ch * H).astype(np.int16)

        assert idx_slot.min() >= 0 and colv.min() >= 0
        filled = perm >= 0
        ef_slot = np.zeros((cfg.e_slots, D), dtype=BF16)
        ef_slot[filled] = ef_bf[perm[filled]]

        in_maps.append(
            {
                "nfn": nfn,
                "nfp": nfp,
                "w": w_bf,
                "bvec": bv,
                "ident": ident,
                "idxr": _wrap16(idx_slot),
                "idxc": _wrap16(colv),
                "eft": np.ascontiguousarray(ef_slot.T),
            }
        )
        perms.append(perm)
    return in_maps, perms


def unpack_out(cfg: Cfg, o, perm, e_core):
    slots = np.asarray(o).T.astype(np.float32)
    res = np.empty((e_core, D), dtype=np.float32)
    filled = perm >= 0
    res[perm[filled]] = slots[filled]
    return res


_CACHE = {}


def kernel(node_features, edge_index, edge_features, W, b):
    from concourse.bass_utils import run_bass_kernel_spmd

    cfg = derive_cfg(edge_index, N_CORES)
    key = (cfg.pair_cap, cfg.single_cap)
    if key not in _CACHE:
        _CACHE[key] = build_nc(cfg)
    nc = _CACHE[key]

    in_maps, perms = make_in_maps(
        cfg, node_features, edge_index, edge_features, W, b, N_CORES
    )
    res = run_bass_kernel_spmd(nc, in_maps, core_ids=list(range(N_CORES)))
    e_core = np.asarray(edge_index).shape[1] // N_CORES
    outs = [
        unpack_out(cfg, res.results[i]["out"], perms[i], e_core)
        for i in range(N_CORES)
    ]
    return np.concatenate(outs, axis=0)
